# revision 86
# baseline (speedup 1.0000x reference)
"""CFConv (SchNet continuous-filter convolution) — Bass/Tile kernel for
8 Trainium2 NeuronCores.

Contract: kernel(**inputs) takes the FULL unsharded inputs (as produced by
reference.setup_inputs()) and returns the FULL [16, 256, 128] f32 output.

Sharding: data-parallel over the batch dim (B=16 -> 2 batches per core).
Host-side work is limited to layout/sharding transforms (slicing per core,
axis transposes of f_ij / x, bf16 pre-cast, int index reformatting); all
arithmetic runs on the NeuronCores.

Per-core kernel (F-on-partitions layout; rows r = atom*64 + neighbor):
  y table:  psY[f, a] = w_in2f.T @ x_t                      (PE)
  cutoff:   s = (0.5 cos(pi r/5)+0.5)(r<5)*mask — cos via a degree-4
            polynomial in r^2 on DVE (no ACT trig-table load); the row
            is written to DRAM scratch and re-read per chunk with a
            0-stride broadcast AP into [128, CHUNK] tiles (DMA only)
  mm1:      psH[h, r] = fw1.T @ f_ij_t                      (PE, bf16)
  ssp:      eH = Exp(psH + fb1); H = Ln(0.5*eH + 0.5)       (ACT; both
            functions steered to the shared natural_log_exp table so the
            1.3us ACT table reload happens once, not per chunk)
  mm2:      psW[f, r] = fw2.T @ H                           (PE)
  gather:   ynbh[f, r] = ytab[f, neighbors(r)]              (GPSIMD)
  product:  t = (psW + fb2) * ynbh  (DVE STT, PSUM-rate pass)
            P = t * sbc             (DVE TT, all-bf16 4x-rate)
  reduce+out: psO[o, 32-atom cols] += wout.T @ P[:, n-slice] per chunk
  out = ssp(psO + b_f2out) stored as [F, A]; host transposes to [A, F].

Modeled (TRN2 cost model) asymptotic steady-state: ~65.4 us/core vs
~179.4 us for the previous Exp/Ln-interleaved kernel (2.7x).  The ACT
engine is ~99% utilized; ~65 us is the floor for this decomposition (the
two ssp table passes are 0.833 ns/elem x 2 x 4.2M elems/core plus
PSUM-bank-capped instruction overheads, irreducible on this HW).
"""
import math
import types
import numpy as np
import bass_rust
import concourse.bass as bass
import concourse.bacc as bacc
import concourse.mybir as mybir
from concourse import tile
from concourse import bass2jax
from concourse.hw_specs import get_activation_tables


def _steered_act_table_loads(self):
    """Table-load insertion with a steered table list.

    The stock pass greedily assigns each activation the FIRST table
    containing its function, which puts Exp in `exp_and_others` and Ln in
    `natural_log` and reloads the ACT table (1.3us) on every Exp<->Ln
    switch.  `natural_log_exp_and_others` holds BOTH.  Emptying the
    function sets of the tables we don't want chosen (list positions are
    the act_func_set_ids, so order must be preserved) steers Exp and Ln
    to the shared table: one load per function-set change instead of ~36.
    """
    has_activation = any(
        isinstance(i, mybir.InstActivation)
        for b in self.main_func.blocks
        for i in b.instructions
    )
    if not has_activation:
        return
    keep = {"natural_log_exp_and_others", "trig_and_small"}
    tables = [
        (name, (s if name in keep else set()))
        for name, s in get_activation_tables(self.m.arch).items()
    ]
    bass_rust.insert_act_table_loads(self, tables)

f32 = mybir.dt.float32
bf16 = mybir.dt.bfloat16
i16 = mybir.dt.int16
i32 = mybir.dt.int32
AF = mybir.ActivationFunctionType
ALU = mybir.AluOpType

B, A, N, G, F = 16, 256, 64, 64, 128
R = A * N
CHUNK = 2048
NCH = R // CHUNK
CUTOFF = 5.0
N_CORES = 8
BPC = B // N_CORES
LN2 = math.log(2.0)


def _host_prep(inputs, n_cores=N_CORES):
    import ml_dtypes
    bpc = BPC
    f_ij = np.asarray(inputs["f_ij"], np.float32)
    fij_b = np.ascontiguousarray(
        f_ij.reshape(B, R, G).transpose(0, 2, 1)).astype(ml_dtypes.bfloat16)
    x = np.asarray(inputs["x"], np.float32)
    x_b = np.ascontiguousarray(
        x.transpose(0, 2, 1)).astype(ml_dtypes.bfloat16)  # [B, F, A]
    nbr_flat = np.asarray(inputs["neighbors"]).reshape(B, R).astype(np.int16)
    wrapped = nbr_flat.reshape(B, R // 16, 16).transpose(0, 2, 1)
    idx = np.ascontiguousarray(np.tile(wrapped, (1, 8, 1)).astype(np.int16))
    r_ij = np.ascontiguousarray(np.asarray(inputs["r_ij"], np.float32))
    mask = np.ascontiguousarray(np.asarray(inputs["pairwise_mask"], np.float32))
    sh = lambda t, c: np.ascontiguousarray(t[c * bpc:(c + 1) * bpc])
    w = lambda k: np.ascontiguousarray(np.asarray(inputs[k], np.float32))
    in_maps = []
    for c in range(n_cores):
        in_maps.append({
            "fij_t": sh(fij_b, c), "x_t": sh(x_b, c), "r_ij": sh(r_ij, c),
            "mask": sh(mask, c), "idx": sh(idx, c),
            "fw1": w("fw1"), "fb1": w("fb1"), "fw2": w("fw2"), "fb2": w("fb2"),
            "w_in2f": w("w_in2f"), "w_f2out": w("w_f2out"),
            "b_f2out": w("b_f2out"),
        })
    return in_maps


def build_nc(bpc=BPC, num_devices=N_CORES, reps=1):
    nc = bacc.Bacc("TRN2", target_bir_lowering=False, debug=False,
                   num_devices=num_devices)
    nc.insert_act_table_loads = types.MethodType(_steered_act_table_loads, nc)
    D = nc.declare_dram_parameter
    fij_t_d = D("fij_t", [bpc, G, R], bf16, isOutput=False)
    xt_d = D("x_t", [bpc, F, A], bf16, isOutput=False)
    r_d = D("r_ij", [bpc, A, N], f32, isOutput=False)
    m_d = D("mask", [bpc, A, N], f32, isOutput=False)
    idx_d = D("idx", [bpc, 128, R // 16], i16, isOutput=False)
    fw1_d = D("fw1", [G, F], f32, isOutput=False)
    fb1_d = D("fb1", [F], f32, isOutput=False)
    fw2_d = D("fw2", [F, F], f32, isOutput=False)
    fb2_d = D("fb2", [F], f32, isOutput=False)
    win_d = D("w_in2f", [F, F], f32, isOutput=False)
    wout_d = D("w_f2out", [F, F], f32, isOutput=False)
    bo_d = D("b_f2out", [F], f32, isOutput=False)
    out_d = D("out", [bpc, F, A], f32, isOutput=True)
    # DRAM scratch for the cutoff row: written once per batch, then re-read
    # with a 0-stride broadcast AP to replicate it across 128 partitions.
    sscr_d = D("s_scr", [bpc, R], bf16, isOutput=True)

    with tile.TileContext(nc) as tc:
        with tc.tile_pool(name="const", bufs=1) as cpool, \
             tc.tile_pool(name="work", bufs=3) as wpool, \
             tc.tile_pool(name="sbb", bufs=2) as sbb, \
             tc.tile_pool(name="pch", bufs=3) as pch, \
             tc.tile_pool(name="ps_h", bufs=2, space="PSUM") as ph, \
             tc.tile_pool(name="ps_w", bufs=2, space="PSUM") as pw, \
             tc.tile_pool(name="ps_t", bufs=1, space="PSUM") as pt, \
             tc.tile_pool(name="ps_o", bufs=1, space="PSUM") as po:

            # ---- constants ----
            fw1_sb = cpool.tile([G, F], bf16, tag="fw1")
            nc.gpsimd.dma_start(out=fw1_sb[:], in_=fw1_d[:, :])
            fw2_sb = cpool.tile([F, F], bf16, tag="fw2")
            nc.gpsimd.dma_start(out=fw2_sb[:], in_=fw2_d[:, :])
            win_sb = cpool.tile([F, F], bf16, tag="win")
            nc.gpsimd.dma_start(out=win_sb[:], in_=win_d[:, :])
            wout_sb = cpool.tile([F, F], bf16, tag="wout")
            nc.gpsimd.dma_start(out=wout_sb[:], in_=wout_d[:, :])
            fb1_sb = cpool.tile([F, 1], f32, tag="fb1")
            nc.sync.dma_start(out=fb1_sb[:],
                              in_=fb1_d.rearrange("(p o) -> p o", o=1))
            fb2_sb = cpool.tile([F, 1], f32, tag="fb2")
            nc.sync.dma_start(out=fb2_sb[:],
                              in_=fb2_d.rearrange("(p o) -> p o", o=1))
            bo_sb = cpool.tile([F, 1], f32, tag="bo")
            nc.sync.dma_start(out=bo_sb[:],
                              in_=bo_d.rearrange("(p o) -> p o", o=1))
            half_sb = cpool.tile([128, 1], f32, tag="half")
            nc.gpsimd.memset(half_sb[:], 0.5)

            for rep in range(reps):
                # ---- per-batch prologue: y table, cutoff row, broadcast ----
                ytabs, sbcs, idxs = [], [], []
                for b in range(bpc):
                    r_sb = wpool.tile([128, 128], f32, tag="rin")
                    nc.sync.dma_start(
                        out=r_sb[:],
                        in_=r_d[b].rearrange("(p q) n -> p (q n)", p=128))
                    m_sb = wpool.tile([128, 128], f32, tag="min")
                    nc.sync.dma_start(
                        out=m_sb[:],
                        in_=m_d[b].rearrange("(p q) n -> p (q n)", p=128))
                    xt_sb = wpool.tile([128, A], bf16, tag="xt")
                    nc.sync.dma_start(out=xt_sb[:], in_=xt_d[b])
                    psY = pt.tile([128, A], f32, tag="tp")
                    nc.tensor.matmul(psY[:], win_sb[:], xt_sb[:],
                                     start=True, stop=True)
                    ytab_sb = sbb.tile([128, A], f32, tag="ytab", bufs=3)
                    nc.vector.tensor_copy(ytab_sb[:], psY[:])
                    ytabs.append(ytab_sb)
                    # C = 0.5*cos(pi*r/5) + 0.5 for r<5, via a degree-4
                    # polynomial in s=r^2 on DVE (max err ~6e-5; keeps the
                    # ACT engine free for Exp/Ln and avoids the trig-table
                    # load).  Coefficients fold in the 0.5*(..)+0.5.
                    c_sb = wpool.tile([128, 128], f32, tag="c")
                    r2_sb = wpool.tile([128, 128], f32, tag="r2")
                    nc.vector.tensor_tensor(r2_sb[:], r_sb[:], r_sb[:],
                                            ALU.mult)
                    CC = (0.999985546, -0.098663983, 0.0032357189,
                          -4.13506240e-05, 2.31550894e-07)
                    nc.vector.tensor_scalar(c_sb[:], r2_sb[:], CC[4], CC[3],
                                            ALU.mult, ALU.add)
                    for coef in (CC[2], CC[1], CC[0]):
                        nc.vector.tensor_tensor(c_sb[:], c_sb[:], r2_sb[:],
                                                ALU.mult)
                        nc.vector.tensor_scalar(c_sb[:], c_sb[:], coef, None,
                                                ALU.add)
                    cut_sb = wpool.tile([128, 128], f32, tag="cut")
                    nc.vector.tensor_scalar(cut_sb[:], r_sb[:], CUTOFF, None,
                                            ALU.is_lt)
                    nc.vector.tensor_tensor(c_sb[:], c_sb[:], cut_sb[:],
                                            ALU.mult)
                    s_bf = wpool.tile([128, 128], bf16, tag="sbf")
                    nc.vector.tensor_tensor(s_bf[:], c_sb[:], m_sb[:],
                                            ALU.mult)
                    # write the cutoff row to DRAM scratch; chunk loop below
                    # re-reads it with a 0-stride broadcast AP
                    nc.sync.dma_start(
                        out=sscr_d[b].rearrange("(p q) -> p q", p=128),
                        in_=s_bf[:])
                    sbcs.append(None)

                    idx_sb = wpool.tile([128, R // 16], i16, tag="idx")
                    nc.sync.dma_start(out=idx_sb[:], in_=idx_d[b])
                    idxs.append(idx_sb)

                # ---- main loops ----
                def emit_reduce(P_prev, c_prev, psO_prev):
                    # accumulate chunk c_prev's 32 atoms into psO cols
                    Pv = P_prev.rearrange("p (a n) -> p n a", n=N)
                    ac = slice(c_prev * (CHUNK // N),
                               (c_prev + 1) * (CHUNK // N))
                    for n in range(N):
                        nc.tensor.matmul(
                            psO_prev[:, ac], wout_sb[:], Pv[:, n, :],
                            start=(n == 0), stop=(n == N - 1))

                def emit_epilogue(psO_prev, b_prev):
                    # ssp + store ([F,A]; host transposes)
                    eo_sb = wpool.tile([128, A], f32, tag="eo")
                    nc.scalar.activation(eo_sb[:], psO_prev[:], AF.Exp,
                                         bias=bo_sb[:], scale=1.0)
                    o_sb = wpool.tile([128, A], f32, tag="o")
                    nc.scalar.activation(o_sb[:], eo_sb[:], AF.Ln,
                                         bias=half_sb[:], scale=0.5)
                    nc.sync.dma_start(out=out_d[b_prev], in_=o_sb[:])

                # The reduce matmuls for a chunk are emitted AFTER the next
                # chunk's mm1 (across batch boundaries too), so the PE FIFO
                # never makes the next Exp wait on the previous chunk's DVE
                # tail.  A batch's epilogue follows its deferred last reduce.
                pending = None  # (P_sb, c, psO, b)
                for b in range(bpc):
                    ytab_sb, idx_sb = ytabs[b], idxs[b]
                    psO = po.tile([128, A], f32, tag="psO")
                    for c in range(NCH):
                        cs = slice(c * CHUNK, (c + 1) * CHUNK)
                        fij_sb = wpool.tile([G, CHUNK], bf16, tag="fij",
                                            bufs=3)
                        nc.sync.dma_start(out=fij_sb[:], in_=fij_t_d[b, :, cs])
                        H_sb = wpool.tile([128, CHUNK], bf16, tag="H", bufs=3)
                        eH_sb = wpool.tile([128, CHUNK], f32, tag="eH",
                                           bufs=3)
                        for j in range(CHUNK // 1024):
                            js = slice(j * 1024, (j + 1) * 1024)
                            psH = ph.tile([128, 1024], f32, tag="psH")
                            for k in range(2):
                                ks = slice(k * 512, (k + 1) * 512)
                                jks = slice(j * 1024 + k * 512,
                                            j * 1024 + (k + 1) * 512)
                                nc.tensor.matmul(psH[:, ks], fw1_sb[:],
                                                 fij_sb[:, jks],
                                                 start=True, stop=True)
                            nc.scalar.activation(eH_sb[:, js], psH[:], AF.Exp,
                                                 bias=fb1_sb[:], scale=1.0)
                        nc.scalar.activation(H_sb[:], eH_sb[:], AF.Ln,
                                             bias=half_sb[:], scale=0.5)
                        if pending is not None:
                            emit_reduce(*pending[:3])
                            if pending[3] != b:
                                emit_epilogue(pending[2], pending[3])
                            pending = None
                        ynbh_sb = wpool.tile([128, CHUNK], f32, tag="ynbh",
                                             bufs=3)
                        nc.gpsimd.ap_gather(
                            out_ap=ynbh_sb.unsqueeze(2),
                            in_ap=ytab_sb.unsqueeze(2),
                            idxs_ap=idx_sb[:, c * (CHUNK // 16):
                                           (c + 1) * (CHUNK // 16)],
                            channels=128, num_elems=A, d=1, num_idxs=CHUNK)
                        sbc_sb = wpool.tile([128, CHUNK], bf16, tag="sbc",
                                            bufs=3)
                        nc.sync.dma_start(
                            out=sbc_sb[:],
                            in_=sscr_d[b, cs].partition_broadcast(128))
                        P_sb = pch.tile([128, CHUNK], bf16, tag="P", bufs=3)
                        t_sb = wpool.tile([128, CHUNK], bf16, tag="t",
                                          bufs=2)
                        for k in range(CHUNK // 512):
                            js = slice(k * 512, (k + 1) * 512)
                            psW = pw.tile([128, 512], f32, tag="psW")
                            nc.tensor.matmul(psW[:], fw2_sb[:],
                                             H_sb[:, js],
                                             start=True, stop=True)
                            nc.vector.scalar_tensor_tensor(
                                out=t_sb[:, js],
                                in0=psW[:], scalar=fb2_sb[:],
                                in1=ynbh_sb[:, js],
                                op0=ALU.add, op1=ALU.mult)
                        nc.vector.tensor_tensor(
                            P_sb[:], t_sb[:], sbc_sb[:], ALU.mult)
                        pending = (P_sb, c, psO, b)
                emit_reduce(*pending[:3])
                emit_epilogue(pending[2], pending[3])
                pending = None
    nc.compile()
    return nc


_NC_CACHE = {}


def kernel(**inputs) -> np.ndarray:
    in_maps = _host_prep(inputs)
    if "nc" not in _NC_CACHE:
        _NC_CACHE["nc"] = build_nc(bpc=BPC, num_devices=N_CORES, reps=1)
    nc = _NC_CACHE["nc"]
    results = bass2jax.run_bass_via_pjrt(nc, in_maps, n_cores=N_CORES)
    out = np.concatenate([r["out"] for r in results], axis=0)  # [B, F, A]
    return np.ascontiguousarray(out.transpose(0, 2, 1)).astype(np.float32)


if __name__ == "__main__":
    rng = np.random.default_rng(0)
    demo = {
        "x": rng.standard_normal((B, A, F)).astype(np.float32),
        "r_ij": (rng.random((B, A, N)) * 6.0).astype(np.float32),
        "f_ij": rng.random((B, A, N, G)).astype(np.float32),
        "neighbors": rng.integers(0, A, (B, A, N)).astype(np.int64),
        "pairwise_mask": (rng.random((B, A, N)) < 0.9).astype(np.float32),
        "fw1": (rng.standard_normal((G, F)) / math.sqrt(G)).astype(np.float32),
        "fb1": np.zeros(F, np.float32),
        "fw2": (rng.standard_normal((F, F)) / math.sqrt(F)).astype(np.float32),
        "fb2": np.zeros(F, np.float32),
        "w_in2f": (rng.standard_normal((F, F)) / math.sqrt(F)).astype(np.float32),
        "w_f2out": (rng.standard_normal((F, F)) / math.sqrt(F)).astype(np.float32),
        "b_f2out": np.zeros(F, np.float32),
    }
    out = kernel(**demo)
    print("kernel output:", out.shape, out.dtype, float(np.abs(out).max()))
